# revision 1
# baseline (speedup 1.0000x reference)
"""Trainium2 Bass kernel for DeBERTa-style disentangled self-attention.

Problem: B=4, S=1024, H=1024, NH=16, HD=64, BUCKETS=256 (span 256).

Sharding: 8 cores <-> (batch b = core//2, head-group hg = core%2 of 8 heads).
Each core computes output[b][:, hg*512:(hg+1)*512].

Math (per b,h), all verified against the reference in numpy:
  term1[q,k] (c2p) = Q[q] . pos_k[Fc(q-k)]
  term2[q,k] (p2c) = K[k] . pos_q[Fc(q-k)]
  Fc(d) = clip(log_bucket(d) + 256, 0, 511)
Both are Toeplitz gathers. We expand the positional tables along relative
distance j (PKE1[j] = pos_k[Fc(1023-j)], PQE2[j] = pos_q[Fc(j-1023)]),
compute per-128-row-tile window matmuls Z[i, w] = X[tile*128+i] . T[w + 896 - 128*tile],
bounce Z through DRAM (contiguous writes, row pitch 1153), and re-read with the
"music-transformer skew" access pattern [[1152, 128], [1, 1056]]: row i, col k
reads Z[i, k-i+127]. The pitch/offset are chosen so every skew descriptor has a
64B-aligned start and stride (2304B) - misaligned skew DMAs were ~20x slower.

Scores are assembled transposed (S^T[k, q]) in PSUM:
  content   : matmul(lhsT=KT, rhs=QT)            (contraction over hd=64)
  p2c term  : identity-matmul accumulate of the skew-read Z2 tile (already [k,q])
  c2p term  : PE-transpose of skew-read Z1 tiles ([q,k] -> [k,q]) into a
              separate bf16 PSUM tile, then added on DVE together with the
              additive mask (-30000 on masked positions)
  softmax   : exp((psum + term1 + mask)/sqrt(192)) WITHOUT max-subtraction
              (logits are O(5); exact up to fp rounding), normalization
              deferred: numer^T goes straight into PV, denominator comes from
              an appended ones-column on V (V_aug[:, 64] = 1).

Scheduling notes (each measured on HW):
  - Z window tiles for head ih+1 are emitted two-per-k-tile INSIDE head ih's
    strip loop: PE is strict FIFO, and a contiguous block of PSUM-eviction-
    gated Z matmuls would stall later strip matmuls behind it.
  - Z evictions alternate ACT/DVE 1:1 (an ACT-heavy split gates the stream).
  - PV accumulates k-tile PAIRS in PSUM before the DVE eviction: each output
    chunk's 2-matmul group closes before the next chunk's start=True clears
    the bank's has_written bits, so the groups don't corrupt each other.
  - Do NOT replace the PE transposes with dma_start_transpose: the xbar
    DMATranspose<->DMACopy mode-transition serialization measured 3x slower.
"""

import math

import numpy as np
import ml_dtypes

import concourse.bass as bass
from concourse import bacc
import concourse.tile as tile
import concourse.mybir as mybir
from concourse.bass_utils import run_bass_kernel_spmd
from concourse.masks import make_identity

BF = ml_dtypes.bfloat16
F32 = np.float32

B, S, H = 4, 1024, 1024
NH, HD = 16, 64
SPAN = 256
MID = 128
MAX_POS = 1024
N_CORES = 8
HEADS_PER_CORE = 8
HG_W = HEADS_PER_CORE * HD          # 512 columns per core
SCALE = math.sqrt(HD * 3)           # sqrt(192)
MASK_ADD = -30000.0                 # additive mask; exp((x-30000)/scale) == 0
WIN = 1152                          # Z window width per 128-row tile
HPAD = 1152                         # padded contraction axis (9*128); row H = ones/bias
NKC = HPAD // 128                   # contraction chunks for projections
ZPITCH = WIN + 1                    # DRAM row pitch; skew stride = ZPITCH-1 = 1152
SKO = 31                            # skew-read left pad so descriptor starts align
SKW = SKO + S + 1                   # skew-read width (1056)
#   elements = 2304 B, a 64 B multiple -> aligned DMA bursts on the skew read


def _log_bucket(rel):
    rel = np.asarray(rel)
    sign = np.sign(rel)
    abs_pos = np.where((rel < MID) & (rel > -MID), MID - 1, np.abs(rel)).astype(np.float64)
    log_pos = np.ceil(np.log(abs_pos / MID) / np.log((MAX_POS - 1) / MID) * (MID - 1)) + MID
    return np.where(abs_pos <= MID, rel.astype(np.float64), log_pos * sign).astype(np.int64)


def _fc(d):
    return np.clip(_log_bucket(d) + SPAN, 0, 2 * SPAN - 1)


def _run_groups(m):
    """Decompose a monotone index map m[j] (j in [0,2048)) into groups
    (j0, p0, nb, L): m[j0 + b*L + l] == p0 + b for b<nb, l<L."""
    m = np.asarray(m)
    starts = np.r_[0, np.nonzero(np.diff(m))[0] + 1]
    lens = np.r_[np.diff(starts), len(m) - starts[-1]]
    vals = m[starts]
    groups = []
    g0 = 0
    for r in range(1, len(starts) + 1):
        if r == len(starts) or lens[r] != lens[g0]:
            groups.append((int(starts[g0]), int(vals[g0]), int(r - g0), int(lens[g0])))
            g0 = r
    for (j0, p0, nb, L) in groups:
        assert np.all(m[j0:j0 + nb * L] == p0 + np.repeat(np.arange(nb), L))
    return groups


_JJ = np.arange(2048)
_IDX1 = _fc(1023 - _JJ)   # PKE1[j] = pos_k[_IDX1[j]]  (c2p)
_IDX2 = _fc(_JJ - 1023)   # PQE2[j] = pos_q[_IDX2[j]]  (p2c)

_BASS_CACHE = None


def _build_bass():
    dt = mybir.dt
    nc = bacc.Bacc("TRN2", target_bir_lowering=False, debug=False,
                   enable_asserts=False, num_devices=N_CORES)

    def inp(name, shape, dtype):
        return nc.dram_tensor(name, shape, dtype, kind="ExternalInput").ap()

    # contraction axis padded to HPAD rows: row H holds the ones/bias rank-1
    # pair (bias folded into weights), rows H+1.. are zero.
    hT = inp("hT", [HPAD, S], dt.bfloat16)           # [hidden^T; ones; 0] for this batch
    wqT = inp("wqT", [HPAD, HG_W], dt.bfloat16)      # [Wq^T; bq; 0] head-group columns
    wkT = inp("wkT", [HPAD, HG_W], dt.bfloat16)
    wvT = inp("wvT", [HPAD, HG_W], dt.bfloat16)
    posTe1 = inp("posTe1", [HPAD, 2048], dt.bfloat16)  # [expanded rel_emb^T; ones; 0]
    posTe2 = inp("posTe2", [HPAD, 2048], dt.bfloat16)
    maskT = inp("maskT", [S, S], dt.bfloat16)        # additive mask, [k, q] layout
    # per-head contiguous output [head, s, hd]; host interleaves columns
    out = nc.dram_tensor("out", [HEADS_PER_CORE, S, HD], dt.float32,
                         kind="ExternalOutput").ap()

    AF = mybir.ActivationFunctionType
    ALU = mybir.AluOpType

    with tile.TileContext(nc) as tc:
        from contextlib import ExitStack
        with ExitStack() as ctx:
            persist = ctx.enter_context(tc.tile_pool(name="persist", bufs=1))
            dram = ctx.enter_context(tc.tile_pool(name="dram", bufs=4, space="DRAM"))

            # ---------------- persistent tiles ----------------
            qt_sb = [persist.tile([128, S], dt.bfloat16, tag=f"qt{i}", name=f"qt{i}") for i in range(4)]
            kt_sb = [persist.tile([128, S], dt.bfloat16, tag=f"kt{i}", name=f"kt{i}") for i in range(4)]
            vaug = [persist.tile([128, HEADS_PER_CORE, HD + 1], dt.bfloat16, tag=f"va{i}", name=f"va{i}")
                    for i in range(8)]
            pke_sb = [persist.tile([128, 2048], dt.bfloat16, tag=f"pke{i}", name=f"pke{i}") for i in range(4)]
            pqe_sb = [persist.tile([128, 2048], dt.bfloat16, tag=f"pqe{i}", name=f"pqe{i}") for i in range(4)]
            mask_sb = [persist.tile([128, S], dt.bfloat16, tag=f"mk{i}", name=f"mk{i}") for i in range(8)]
            ident = persist.tile([128, 128], dt.bfloat16, tag="ident", name="ident")
            make_identity(nc, ident)
            for i in range(8):
                nc.sync.dma_start(out=mask_sb[i], in_=maskT[128 * i:128 * (i + 1), :])

            # ---------------- projections (scoped pools) ----------------
            with ExitStack() as pctx:
                ppool = pctx.enter_context(tc.tile_pool(name="proj", bufs=1))
                ppsum = pctx.enter_context(tc.tile_pool(name="ppsum", bufs=4, space="PSUM"))
                h_sb = [ppool.tile([128, S], dt.bfloat16, tag=f"h{i}", name=f"h{i}") for i in range(NKC)]
                wq_sb = [ppool.tile([128, HG_W], dt.bfloat16, tag=f"wq{i}", name=f"wq{i}") for i in range(NKC)]
                wk_sb = [ppool.tile([128, HG_W], dt.bfloat16, tag=f"wk{i}", name=f"wk{i}") for i in range(NKC)]
                wv_sb = [ppool.tile([128, HG_W], dt.bfloat16, tag=f"wv{i}", name=f"wv{i}") for i in range(NKC)]
                for i in range(NKC):
                    sl = slice(128 * i, 128 * (i + 1))
                    nc.sync.dma_start(out=h_sb[i], in_=hT[sl, :])
                    nc.sync.dma_start(out=wq_sb[i], in_=wqT[sl, :])
                    nc.sync.dma_start(out=wk_sb[i], in_=wkT[sl, :])
                    nc.sync.dma_start(out=wv_sb[i], in_=wvT[sl, :])

                # QT / KT (transposed layouts [hd, s])
                for (w_sb, q_dst) in ((wq_sb, qt_sb), (wk_sb, kt_sb)):
                    for pt in range(4):
                        for sh in range(2):
                            ps = ppsum.tile([128, 512], dt.float32, tag="pp", name="pp")
                            for hc in range(NKC):
                                nc.tensor.matmul(
                                    out=ps,
                                    lhsT=w_sb[hc][:, 128 * pt:128 * (pt + 1)],
                                    rhs=h_sb[hc][:, 512 * sh:512 * (sh + 1)],
                                    start=(hc == 0), stop=(hc == NKC - 1))
                            nc.scalar.copy(
                                out=q_dst[pt][:, 512 * sh:512 * (sh + 1)], in_=ps)

                # V (straight layout [s, hd]) + ones column
                for st in range(8):
                    ps = ppsum.tile([128, 512], dt.float32, tag="pp", name="pp")
                    for hc in range(NKC):
                        nc.tensor.matmul(
                            out=ps,
                            lhsT=h_sb[hc][:, 128 * st:128 * (st + 1)],
                            rhs=wv_sb[hc],
                            start=(hc == 0), stop=(hc == NKC - 1))
                    nc.vector.tensor_copy(
                        out=vaug[st][:, :, 0:HD],
                        in_=ps.rearrange("p (h d) -> p h d", h=HEADS_PER_CORE))
                    nc.vector.memset(vaug[st][:, :, HD:HD + 1], 1.0)

                # positional tables: project the HOST-expanded rel embeddings
                # (gather fused into the projection matmul; tables land
                # directly in their persistent SBUF pair-tiles)
                for (src_e, w_sb, dst_tab) in (
                        (posTe1, wk_sb, pke_sb), (posTe2, wq_sb, pqe_sb)):
                    pe_t = [ppool.tile([128, 2048], dt.bfloat16, tag=f"pe{i}",
                                       name=f"pe_{i}") for i in range(NKC)]
                    for i in range(NKC):
                        nc.sync.dma_start(out=pe_t[i], in_=src_e[128 * i:128 * (i + 1), :])
                    for pt in range(4):
                        for c4 in range(4):
                            ps = ppsum.tile([128, 512], dt.float32, tag="pp", name="pp_pos")
                            for hc in range(NKC):
                                nc.tensor.matmul(
                                    out=ps,
                                    lhsT=w_sb[hc][:, 128 * pt:128 * (pt + 1)],
                                    rhs=pe_t[hc][:, 512 * c4:512 * (c4 + 1)],
                                    start=(hc == 0), stop=(hc == NKC - 1))
                            nc.scalar.copy(
                                out=dst_tab[pt][:, 512 * c4:512 * (c4 + 1)], in_=ps)

            # ---------------- main per-head pipeline ----------------
            zpsum = ctx.enter_context(tc.tile_pool(name="zpsum", bufs=2, space="PSUM"))
            spsum = ctx.enter_context(tc.tile_pool(name="spsum", bufs=2, space="PSUM"))
            trpsum = ctx.enter_context(tc.tile_pool(name="trpsum", bufs=2, space="PSUM"))
            pvpsum = ctx.enter_context(tc.tile_pool(name="pvpsum", bufs=2, space="PSUM"))
            zsb_p = ctx.enter_context(tc.tile_pool(name="zsb", bufs=4))
            t1_p = ctx.enter_context(tc.tile_pool(name="t1", bufs=16))
            t2_p = ctx.enter_context(tc.tile_pool(name="t2", bufs=4))
            pre_p = ctx.enter_context(tc.tile_pool(name="pre", bufs=8))
            nm_p = ctx.enter_context(tc.tile_pool(name="nm", bufs=6))
            acc_p = ctx.enter_context(tc.tile_pool(name="acc", bufs=2))
            sml_p = ctx.enter_context(tc.tile_pool(name="sml", bufs=4))

            zdram = {}

            def alloc_z(ih):
                z1 = dram.tile([8, 128, ZPITCH], dt.bfloat16, tag="z1", name="z1")
                z2 = dram.tile([8, 128, ZPITCH], dt.bfloat16, tag="z2", name="z2")
                zdram[ih] = (z1, z2)
                return z1, z2

            def emit_z_tile(ih, zd, zi, t):
                """One Z window tile (source zi: 0=c2p/Q, 1=p2c/K) for head ih."""
                pair, half = ih // 2, ih % 2
                lo = 64 * half
                x_sb, tab = ((qt_sb, pke_sb), (kt_sb, pqe_sb))[zi]
                woff = 896 - 128 * t
                zt = zsb_p.tile([128, ZPITCH], dt.bfloat16, tag="zt", name="zt")
                nc.vector.memset(zt[:, WIN:ZPITCH], 0.0)
                for ci, (w0, w1) in enumerate(((0, 512), (512, 1024), (1024, WIN))):
                    ps = zpsum.tile([128, 512], dt.float32, tag="zp", name="zp")
                    nc.tensor.matmul(
                        out=ps[:, 0:w1 - w0],
                        lhsT=x_sb[pair][lo:lo + 64, 128 * t:128 * (t + 1)],
                        rhs=tab[pair][lo:lo + 64, woff + w0:woff + w1],
                        start=True, stop=True)
                    # alternate eviction copies across ACT/DVE (1:1 matches the
                    # matmul rate; an ACT-heavy split gates the Z stream)
                    if (t + ci) % 2 == 0:
                        nc.scalar.copy(out=zt[:, w0:w1], in_=ps[:, 0:w1 - w0])
                    else:
                        nc.vector.tensor_copy(out=zt[:, w0:w1], in_=ps[:, 0:w1 - w0])
                nc.sync.dma_start(out=zd[zi][t], in_=zt)

            def emit_z(ih):
                zd = alloc_z(ih)
                for zi in range(2):
                    for t in range(8):
                        emit_z_tile(ih, zd, zi, t)

            def skew_ap(zd, t):
                sub = zd[t]
                # descriptor starts at +96 elems (192 B) so every row start is
                # 64B-aligned; real data begins at column SKO=31 of the tile
                return bass.AP(tensor=sub.tensor, offset=sub.offset + 127 - SKO,
                               ap=[[ZPITCH - 1, 128], [1, SKW]])

            def emit_strips(ih, znext=None):
                """Score strips + softmax + PV for head ih. When znext is set,
                the next head's Z window tiles are emitted interleaved, two per
                k-tile iteration, so PE alternates Z matmuls (gated by PSUM
                evictions on ACT/DVE) with strip matmuls and never starves."""
                pair, half = ih // 2, ih % 2
                lo = 64 * half
                z1, z2 = zdram.pop(ih)
                znd = alloc_z(znext) if znext is not None else None
                t1sb = []
                for t in range(8):
                    tt = t1_p.tile([128, SKW], dt.bfloat16, tag="t1", name="t1")
                    nc.sync.dma_start(out=tt, in_=skew_ap(z1, t))
                    t1sb.append(tt)
                ctxacc = acc_p.tile([128, 8, HD + 1], dt.float32, tag="acc", name="acc")
                nm_hold = [[None, None], [None, None]]
                for kt in range(8):
                    if znext is not None:
                        emit_z_tile(znext, znd, 0, kt)
                        emit_z_tile(znext, znd, 1, kt)
                    t2sb = t2_p.tile([128, SKW], dt.bfloat16, tag="t2", name="t2")
                    nc.sync.dma_start(out=t2sb, in_=skew_ap(z2, kt))
                    for qh in range(2):
                        qsl = slice(512 * qh, 512 * (qh + 1))
                        sp = spsum.tile([128, 512], dt.float32, tag="sp", name="sp")
                        nc.tensor.matmul(
                            out=sp,
                            lhsT=kt_sb[pair][lo:lo + 64, 128 * kt:128 * (kt + 1)],
                            rhs=qt_sb[pair][lo:lo + 64, qsl],
                            start=True, stop=False)
                        nc.tensor.matmul(out=sp, lhsT=ident,
                                         rhs=t2sb[:, SKO + 512 * qh:SKO + 512 * (qh + 1)],
                                         start=False, stop=True)
                        trp = trpsum.tile([128, 512], dt.bfloat16, tag="trp", name="trp")
                        for c in range(4):
                            qt4 = 4 * qh + c
                            nc.tensor.matmul(
                                out=trp[:, 128 * c:128 * (c + 1)],
                                lhsT=t1sb[qt4][:, SKO + 128 * kt:SKO + 128 * (kt + 1)],
                                rhs=ident, is_transpose=True)
                        t1m = pre_p.tile([128, 512], dt.bfloat16, tag="t1m", name="t1m")
                        nc.vector.tensor_add(out=t1m, in0=trp, in1=mask_sb[kt][:, qsl])
                        pre = pre_p.tile([128, 512], dt.bfloat16, tag="pre", name="pre")
                        nc.vector.scalar_tensor_tensor(
                            out=pre, in0=sp, scalar=1.0, in1=t1m,
                            op0=ALU.mult, op1=ALU.add)
                        nm = nm_p.tile([128, 512], dt.bfloat16, tag="nm", name="nm")
                        nc.scalar.activation(out=nm, in_=pre, func=AF.Exp,
                                             scale=float(1.0 / SCALE))
                        nm_hold[kt % 2][qh] = nm
                    if kt % 2 == 0:
                        continue
                    # PV over a k-tile pair: per chunk, a closed 2-matmul
                    # accumulation group (safe: each group finishes before the
                    # next chunk's start=True clears the bank's has_written)
                    for qh in range(2):
                        pv = pvpsum.tile([128, 4, HD + 1], dt.float32, tag="pv", name="pv")
                        for c in range(4):
                            for ki, kk in enumerate((kt - 1, kt)):
                                nc.tensor.matmul(
                                    out=pv[:, c, :],
                                    lhsT=nm_hold[kk % 2][qh][:, 128 * c:128 * (c + 1)],
                                    rhs=vaug[kk][:, ih, :],
                                    start=(ki == 0), stop=(ki == 1))
                        dst = ctxacc[:, 4 * qh:4 * (qh + 1), :]
                        if kt == 1:
                            nc.vector.tensor_copy(out=dst, in_=pv)
                        else:
                            nc.vector.tensor_add(out=dst, in0=dst, in1=pv)
                # epilogue: normalize + store
                cout = sml_p.tile([128, 8, HD], dt.float32, tag="cout", name="cout")
                rec = sml_p.tile([128, 8], dt.float32, tag="rec", name="rec")
                nc.vector.reciprocal(out=rec, in_=ctxacc[:, :, HD])
                for qc in range(8):
                    nc.vector.tensor_scalar_mul(
                        out=cout[:, qc, :], in0=ctxacc[:, qc, 0:HD],
                        scalar1=rec[:, qc:qc + 1])
                nc.sync.dma_start(
                    out=out[ih].rearrange("(c p) d -> p c d", p=128), in_=cout)

            import os
            n_emit = HEADS_PER_CORE
            # KERNEL_REPEAT repeats the (idempotent) main loop for timing-slope
            # measurement; any setting still produces correct output.
            n_rep = int(os.environ.get("KERNEL_REPEAT", "1"))
            for _rep in range(n_rep):
                emit_z(0)
                for ih in range(n_emit):
                    emit_strips(ih, znext=ih + 1 if ih + 1 < n_emit else None)

    nc.finalize()
    return nc


def _prep_core_inputs(inputs):
    hs = np.asarray(inputs["hidden_states"], dtype=np.float32)
    am = np.asarray(inputs["attention_mask"])
    rel = np.asarray(inputs["rel_embeddings"], dtype=np.float32)
    Wq = np.asarray(inputs["Wq"], dtype=np.float32)
    Wk = np.asarray(inputs["Wk"], dtype=np.float32)
    Wv = np.asarray(inputs["Wv"], dtype=np.float32)
    bq = np.asarray(inputs["bq"], dtype=np.float32)
    bk = np.asarray(inputs["bk"], dtype=np.float32)
    bv = np.asarray(inputs["bv"], dtype=np.float32)

    pos = rel[:2 * SPAN]

    def padrows(mat, extra_row):
        outm = np.zeros((HPAD, mat.shape[1]), np.float32)
        outm[:H] = mat
        outm[H] = extra_row
        return outm.astype(BF)

    posTe1 = padrows(np.ascontiguousarray(pos[_IDX1].T), 1.0)   # [HPAD, 2048]
    posTe2 = padrows(np.ascontiguousarray(pos[_IDX2].T), 1.0)

    in_maps = []
    for c in range(N_CORES):
        b, hg = c // 2, c % 2
        cols = slice(HG_W * hg, HG_W * (hg + 1))
        hT = padrows(np.ascontiguousarray(hs[b].T), 1.0)
        maskadd = np.where(am[b, 0].T == 0, MASK_ADD, 0.0).astype(BF)
        in_maps.append({
            "hT": hT,
            "wqT": padrows(np.ascontiguousarray(Wq.T[:, cols]), bq[cols]),
            "wkT": padrows(np.ascontiguousarray(Wk.T[:, cols]), bk[cols]),
            "wvT": padrows(np.ascontiguousarray(Wv.T[:, cols]), bv[cols]),
            "posTe1": posTe1,
            "posTe2": posTe2,
            "maskT": maskadd,
        })
    return in_maps


def kernel(**inputs):
    global _BASS_CACHE
    if _BASS_CACHE is None:
        _BASS_CACHE = _build_bass()
    nc = _BASS_CACHE
    in_maps = _prep_core_inputs(inputs)
    res = run_bass_kernel_spmd(nc, in_maps, core_ids=list(range(N_CORES)))
    out = np.zeros((B, S, NH * HD), np.float32)
    for c in range(N_CORES):
        b, hg = c // 2, c % 2
        oc = res.results[c]["out"]                    # [8, S, HD]
        out[b, :, HG_W * hg:HG_W * (hg + 1)] = (
            oc.transpose(1, 0, 2).reshape(S, HG_W))
    return out

